# revision 1
# baseline (speedup 1.0000x reference)
"""Trainium2 Bass kernel: MLP-scored masked attention (sparse_attention).

Reference computation per batch b (B=4096, S=200, F=64):
    att_x = concat([q, k, q-k, q*k])            # [S, 256]
    h1 = relu(att_x @ W1 + b1)                  # [S, 80]
    h2 = relu(h1 @ W2 + b2)                     # [S, 40]
    sc = h2 @ W3 + b3                           # [S, 1]
    sc = where(arange(S) < seq_len, sc, NEG_BIG)
    p  = softmax(sc)
    out = p @ k                                 # [1, 64]

Key algebra: with W1 = [W1q; W1k; W1d; W1m] (row blocks of 64),
    att_x @ W1 = q@(W1q+W1d) + k@(W1k-W1d) + (q*k)@W1m
Host precomputes: rhsT = [k^T; (q*k)^T] (the full K=128 moving operand),
A = q@(W1q+W1d)+b1 (per-batch relu1 bias), and the additive NEG_BIG mask.
b3 is softmax-invariant and dropped; 1/sum(exp) is applied to the final
output columns, so the transposed probabilities are raw exp().

Per (b, s) device work: h1 = relu(Ws^T rhs + A) (one K=128 matmul),
h2 = relu(W2^T h1 + b2), sc = W3^T h2, masked softmax over s, out = p @ k.

Distribution: pure data-parallel, batch 4096 sharded over 8 cores (512 each).

Schedule (per tile of 64 batches, pairs p of 2 batches): software-pipelined
so each engine's in-order queue stays busy; iteration i emits
  PE:  h1(i), h2a(i-1), h2b(i-1), sc(i-2)         (+ prev tile's out matmuls)
  ACT: relu1a(i-1), sc-copy(i-2, every 2nd)
  DVE: relu1b(i-1), relu2(i-1)
h2a consumes only the ACT-written h1 half and h2b only the DVE half, so
each carries one producer wait.  The previous tile's softmax + output phase
(exp-transpose + per-batch out matmuls) interleaves as PE gap filler.

Walrus constraint: compute instructions carry at most ONE semaphore wait;
_split_multi_waits hoists extras onto standalone InstDrains.
"""

import numpy as np
import os
import sys

sys.path.insert(0, "/opt/trn_rl_repo")

import ml_dtypes
from concourse import bass, mybir, masks
from concourse.tile import TileContext
from concourse.bass_utils import run_bass_kernel_spmd

BF16 = mybir.dt.bfloat16
F32 = mybir.dt.float32

B, S, F = 4096, 200, 64
H1, H2 = 80, 40
NCORES = 8
BPC = B // NCORES   # 512 batches per core
TILE = 64           # batches per tile
NT = BPC // TILE    # 8 tiles
PAIRS = TILE // 2   # 32 pairs per tile
NEG_BIG = float(-(2**32) + 1)
SPLIT_WAITS = True

OUT_MM_STEPS = 8
OUT_STEPS = 3 + OUT_MM_STEPS + 1


def build_graph():
    nc = bass.Bass()

    keys_e = nc.declare_dram_parameter("keys", [BPC, S, F], BF16, isOutput=False)
    # rows 0:64 = k^T, rows 64:128 = (q*k)^T  (host-precomputed, batch-major)
    rhsT_e = nc.declare_dram_parameter("rhsT", [128, BPC, S], BF16, isOutput=False)
    A_e = nc.declare_dram_parameter("Abias", [H1, BPC], F32, isOutput=False)
    amask_e = nc.declare_dram_parameter("amask", [BPC, S], F32, isOutput=False)
    Ws_e = nc.declare_dram_parameter("Ws", [128, H1], BF16, isOutput=False)
    W2p_e = nc.declare_dram_parameter("W2p", [H1, 64], BF16, isOutput=False)
    W3pp_e = nc.declare_dram_parameter("W3pp", [128, 2], BF16, isOutput=False)
    b2pp_e = nc.declare_dram_parameter("b2pp", [128, 1], F32, isOutput=False)
    out_e = nc.declare_dram_parameter("out", [BPC, F], F32, isOutput=True)
    dbg_e = nc.declare_dram_parameter("dbg", [H1, H1], F32, isOutput=True)

    with TileContext(nc) as tc:
        from contextlib import ExitStack
        with ExitStack() as _es:
            constp = _es.enter_context(tc.tile_pool(name="const", bufs=1))
            p_rhs = _es.enter_context(tc.tile_pool(name="rhs", bufs=2))
            p_kn1 = _es.enter_context(tc.tile_pool(name="kn1", bufs=2))
            p_kn2 = _es.enter_context(tc.tile_pool(name="kn2", bufs=2))
            p_h1a = _es.enter_context(tc.tile_pool(name="h1asb", bufs=4))
            p_h1b = _es.enter_context(tc.tile_pool(name="h1bsb", bufs=4))
            p_h2 = _es.enter_context(tc.tile_pool(name="h2sb", bufs=4))
            p_scores = _es.enter_context(tc.tile_pool(name="scores", bufs=2))
            p_scw = _es.enter_context(tc.tile_pool(name="scw", bufs=2))
            p_scrd = _es.enter_context(tc.tile_pool(name="scrd", bufs=2, space="DRAM"))
            p_soft = _es.enter_context(tc.tile_pool(name="soft", bufs=2))
            p_small = _es.enter_context(tc.tile_pool(name="smalls", bufs=2))
            p_pT = _es.enter_context(tc.tile_pool(name="pTp", bufs=2))
            p_outs = _es.enter_context(tc.tile_pool(name="outs", bufs=2))
            pp_h1 = _es.enter_context(tc.tile_pool(name="ph1", bufs=3, space="PSUM"))
            pp_h2 = _es.enter_context(tc.tile_pool(name="ph2", bufs=2, space="PSUM"))
            pp_sc = _es.enter_context(tc.tile_pool(name="psc", bufs=1, space="PSUM"))
            pp_misc = _es.enter_context(tc.tile_pool(name="pmisc", bufs=1, space="PSUM"))
            pp_out = _es.enter_context(tc.tile_pool(name="pout", bufs=1, space="PSUM"))

            ident = constp.tile([64, 64], F32)
            masks.make_identity(nc, ident[:, :])
            identb = constp.tile([64, 64], BF16)
            nc.vector.tensor_copy(identb[:, :], ident[:, :])
            Ws_sb = constp.tile([128, H1], BF16)
            nc.sync.dma_start(out=Ws_sb[:, :], in_=Ws_e[:, :])
            W2p_sb = constp.tile([H1, 64], BF16)
            nc.sync.dma_start(out=W2p_sb[:, :], in_=W2p_e[:, :])
            W3pp_sb = constp.tile([128, 2], BF16)
            nc.sync.dma_start(out=W3pp_sb[:, :], in_=W3pp_e[:, :])
            b2pp_sb = constp.tile([128, 1], F32)
            nc.sync.dma_start(out=b2pp_sb[:, :], in_=b2pp_e[:, :])
            junk_sb = constp.tile([H1, H1], F32)
            nc.vector.memset(junk_sb[:, :], 0.0)
            # ACT observer: introduce the b2pp DMA queue to ScalarE
            nc.scalar.activation(
                junk_sb[:, 0:1], b2pp_sb[0:H1, :], mybir.ActivationFunctionType.Copy
            )

            # ---- PE semaphore observers: one fresh wait per matmul ----
            jp = pp_misc.tile([H1, H1], F32, tag="ps_misc")
            nc.tensor.transpose(jp[0:64, 0:64], ident[:, :], ident[:, :])  # Pool
            nc.tensor.matmul(jp[0:H1, 0:H1], Ws_sb[:, :], Ws_sb[:, :],
                             start=True, stop=True)                        # Ws DMA q
            nc.tensor.matmul(jp[0:64, 0:64], W2p_sb[:, :], W2p_sb[:, :],
                             start=True, stop=True)                        # W2p DMA q
            nc.tensor.matmul(jp[0:2, 0:2], W3pp_sb[:, :], W3pp_sb[:, :],
                             start=True, stop=True)                        # W3pp DMA q
            nc.vector.tensor_copy(junk_sb[:, :], jp[:, :])

            def tile_prologue(t):
                b0 = t * TILE
                st = {}
                A_sb = p_small.tile([H1, TILE], F32, tag="A", name="A_sb")
                nc.sync.dma_start(out=A_sb[:, :], in_=A_e[:, b0 : b0 + TILE])
                st["A"] = A_sb
                am = p_soft.tile([TILE, S], F32, tag="amask", name="amask_sb")
                nc.sync.dma_start(out=am[:, :], in_=amask_e[b0 : b0 + TILE, :])
                st["amask"] = am

                rhs = p_rhs.tile([128, TILE * S], BF16, name="rhs_all")
                CH = 16
                for c in range(0, TILE, CH):
                    nc.sync.dma_start(
                        out=rhs[:, c * S : (c + CH) * S].rearrange(
                            "p (g s) -> p g s", g=CH
                        ),
                        in_=rhsT_e[:, b0 + c : b0 + c + CH, :],
                    )
                st["rhs"] = rhs

                kn1 = p_kn1.tile([128, TILE * F], BF16, name="kn1")
                kn2 = p_kn2.tile([72, TILE * F], BF16, name="kn2")
                KG = 8
                for j in range(0, TILE, KG):
                    b = b0 + j
                    nc.sync.dma_start(
                        out=kn1[:, j * F : (j + KG) * F].rearrange(
                            "p (g f) -> p g f", g=KG
                        ),
                        in_=keys_e[b : b + KG, 0:128, :].rearrange("g p f -> p g f"),
                    )
                    nc.sync.dma_start(
                        out=kn2[:, j * F : (j + KG) * F].rearrange(
                            "p (g f) -> p g f", g=KG
                        ),
                        in_=keys_e[b : b + KG, 128:S, :].rearrange("g p f -> p g f"),
                    )
                st["kn1"], st["kn2"] = kn1, kn2
                st["scores"] = p_scores.tile([TILE, S], F32, name="scores")
                st["scW"] = p_scw.tile([2, PAIRS * S], F32, name="scW")
                st["t"] = t
                return st

            def emit_h1(st, p):
                # separate PSUM tiles per batch half so relu1a (ACT) and
                # relu1b (DVE) read different tiles — no reader coupling
                h1a_ps = pp_h1.tile([H1, S], F32, tag="h1ps", name="h1a_ps")
                nc.tensor.matmul(
                    h1a_ps[:, :], Ws_sb[:, :],
                    st["rhs"][:, 2 * p * S : (2 * p + 1) * S],
                    start=True, stop=True,
                )
                h1b_ps = pp_h1.tile([H1, S], F32, tag="h1ps", name="h1b_ps")
                nc.tensor.matmul(
                    h1b_ps[:, :], Ws_sb[:, :],
                    st["rhs"][:, (2 * p + 1) * S : (2 * p + 2) * S],
                    start=True, stop=True,
                )
                st[("h1ps", p)] = (h1a_ps, h1b_ps)

            def emit_relu1(st, p):
                h1a_ps, h1b_ps = st.pop(("h1ps", p))
                h1a = p_h1a.tile([H1, S], BF16, name="h1a_sb")
                h1b = p_h1b.tile([H1, S], BF16, name="h1b_sb")
                A = st["A"]
                nc.scalar.activation(
                    h1a[:, :], h1a_ps[:, :],
                    mybir.ActivationFunctionType.Relu,
                    bias=A[:, 2 * p : 2 * p + 1], scale=1.0,
                )
                nc.vector.tensor_scalar(
                    h1b[:, :], h1b_ps[:, :],
                    A[:, 2 * p + 1 : 2 * p + 2], 0.0,
                    mybir.AluOpType.add, mybir.AluOpType.max,
                )
                st[("h1sb", p)] = (h1a, h1b)

            def emit_h2(st, p):
                # pairs 2g and 2g+1 share one single-bank [128, 2S] psum tile
                h1a, h1b = st.pop(("h1sb", p))
                if p % 2 == 0:
                    st["h2ps2"] = pp_h2.tile([128, 2 * S], F32, name="h2_ps")
                h2_ps = st["h2ps2"]
                c0 = (p % 2) * S
                nc.tensor.matmul(
                    h2_ps[0:64, c0 : c0 + S], W2p_sb[:, :], h1a[:, :],
                    start=True, stop=True, tile_position=(0, 0),
                )
                nc.tensor.matmul(
                    h2_ps[64:128, c0 : c0 + S], W2p_sb[:, :], h1b[:, :],
                    start=True, stop=True, tile_position=(0, 64),
                )

            def emit_relu2(st, g):
                h2_ps = st.pop("h2ps2")
                h2_sb = p_h2.tile([128, 2 * S], BF16, name="h2_sb")
                nc.vector.tensor_scalar(
                    h2_sb[:, :], h2_ps[:, :], b2pp_sb[:, 0:1], 0.0,
                    mybir.AluOpType.add, mybir.AluOpType.max,
                )
                st[("h2sb", g)] = h2_sb

            def emit_sc(st, g):
                h2_sb = st.pop(("h2sb", g))
                sc_ps = pp_sc.tile([2, 2 * S], F32, name="sc_ps")
                nc.tensor.matmul(
                    sc_ps[:, :], W3pp_sb[:, :], h2_sb[:, :],
                    start=True, stop=True,
                )
                st[("scps", g)] = sc_ps

            def emit_sccopy(st, g):
                sc_ps = st.pop(("scps", g))
                nc.scalar.activation(
                    st["scW"][:, 2 * g * S : (2 * g + 2) * S], sc_ps[:, :],
                    mybir.ActivationFunctionType.Copy,
                )

            def emit_regroup(st):
                scr = p_scrd.tile([TILE, S], F32, name="scr")
                nc.sync.dma_start(
                    out=scr[:, :].rearrange("(p two) s -> two p s", two=2),
                    in_=st["scW"][:, :].rearrange("two (p s) -> two p s", p=PAIRS),
                )
                nc.sync.dma_start(out=st["scores"][:, :], in_=scr[:, :])

            # ---------------- output phase (softmax + p@k) ----------------
            def emit_out_step(st, step):
                t = st["t"]
                b0 = t * TILE
                if step == 0:
                    maskd = p_soft.tile([TILE, S], F32, tag="maskd", name="maskd")
                    nc.vector.tensor_tensor(
                        maskd[:, :], st["scores"][:, :], st["amask"][:, :],
                        mybir.AluOpType.add,
                    )
                    rmax = p_small.tile([TILE, 1], F32, tag="rmax", name="rmax")
                    nc.vector.tensor_reduce(
                        rmax[:, :], maskd[:, :], mybir.AxisListType.X,
                        mybir.AluOpType.max,
                    )
                    nrmax = p_small.tile([TILE, 1], F32, tag="nrmax", name="nrmax")
                    nc.vector.tensor_scalar_mul(nrmax[:, :], rmax[:, :], -1.0)
                    ex = p_soft.tile([TILE, S], BF16, tag="ex", name="ex")
                    rsum = p_small.tile([TILE, 1], F32, tag="rsum", name="rsum")
                    nc.scalar.activation(
                        ex[:, :], maskd[:, :], mybir.ActivationFunctionType.Exp,
                        bias=nrmax[:, 0:1], scale=1.0, accum_out=rsum[:, 0:1],
                    )
                    rinv = p_small.tile([TILE, 1], F32, tag="rinv", name="rinv")
                    nc.vector.reciprocal(rinv[:, :], rsum[:, :])
                    st["ex"], st["rinv"] = ex, rinv
                elif step == 1:
                    pT_ps = pp_misc.tile([128, TILE], BF16, tag="ps_misc",
                                         name="pT_ps1")
                    nc.tensor.transpose(
                        pT_ps[0:128, 0:TILE], st["ex"][:, 0:128], identb[:, :]
                    )
                    pT1 = p_pT.tile([128, TILE], BF16, tag="pT1", name="pT1")
                    nc.vector.tensor_copy(pT1[:, :], pT_ps[:, :])
                    st["pT1"] = pT1
                elif step == 2:
                    pT_ps = pp_misc.tile([72, TILE], BF16, tag="ps_misc",
                                         name="pT_ps2")
                    nc.tensor.transpose(
                        pT_ps[0:72, 0:TILE], st["ex"][:, 128:S], identb[:, :]
                    )
                    pT2 = p_pT.tile([72, TILE], BF16, tag="pT2", name="pT2")
                    nc.vector.tensor_copy(pT2[:, :], pT_ps[:, :])
                    st["pT2"] = pT2
                    st["outps"] = pp_out.tile([F, TILE], F32, name="out_ps")
                elif step < 3 + OUT_MM_STEPS:
                    g = step - 3
                    n = TILE // OUT_MM_STEPS
                    kn1, kn2 = st["kn1"], st["kn2"]
                    out_ps, pT1, pT2 = st["outps"], st["pT1"], st["pT2"]
                    for j in range(g * n, (g + 1) * n):
                        cj = j * F
                        nc.tensor.matmul(
                            out_ps[:, j : j + 1], kn1[:, cj : cj + F],
                            pT1[:, j : j + 1], start=True, stop=False,
                        )
                        nc.tensor.matmul(
                            out_ps[:, j : j + 1], kn2[:, cj : cj + F],
                            pT2[:, j : j + 1], start=False, stop=True,
                        )
                else:
                    out_ps = st.pop("outps")
                    outT_sb = p_outs.tile([F, TILE], F32, tag="outT", name="outT_sb")
                    nc.vector.tensor_copy(outT_sb[:, :], out_ps[:, :])
                    outF_ps = pp_misc.tile([TILE, F], F32, tag="ps_misc",
                                           name="outF_ps")
                    nc.tensor.transpose(outF_ps[:, :], outT_sb[:, :], ident[:, :])
                    out_sb = p_outs.tile([TILE, F], F32, tag="outf", name="out_sb")
                    nc.vector.tensor_scalar(
                        out_sb[:, :], outF_ps[:, :], st["rinv"][:, 0:1], None,
                        mybir.AluOpType.mult,
                    )
                    nc.sync.dma_start(out=out_e[b0 : b0 + TILE, :], in_=out_sb[:, :])

            # ---------------- main loop ----------------
            prev = None
            OUT_START = 2
            for t in range(NT):
                st = tile_prologue(t)
                out_step = 0
                for i in range(PAIRS + 4):
                    if 1 <= i <= PAIRS:
                        emit_relu1(st, i - 1)
                    if i < PAIRS:
                        emit_h1(st, i)
                    if 1 <= i <= PAIRS:
                        emit_h2(st, i - 1)
                    # relu2(g) once both pairs of group g have h2'd (i-1 = 2g+1)
                    if i >= 2 and (i - 1) % 2 == 1 and (i - 2) // 2 < PAIRS // 2:
                        emit_relu2(st, (i - 2) // 2)
                    # sc(g) one iter later, sccopy(g) one after that
                    if i >= 3 and (i - 3) % 2 == 0 and (i - 3) // 2 < PAIRS // 2:
                        emit_sc(st, (i - 3) // 2)
                    if i >= 4 and (i - 4) % 2 == 0 and (i - 4) // 2 < PAIRS // 2:
                        emit_sccopy(st, (i - 4) // 2)
                    if prev is not None and i >= OUT_START and out_step < OUT_STEPS:
                        emit_out_step(prev, out_step)
                        out_step += 1
                emit_regroup(st)
                prev = st

            for step in range(OUT_STEPS):
                emit_out_step(prev, step)

            nc.sync.dma_start(out=dbg_e[:, :], in_=junk_sb[:, :])

    if SPLIT_WAITS:
        _drop_own_engine_waits(nc)
        _split_multi_waits(nc)
    return nc


# walrus CoreV2/V3 codegen allows only ONE sync-wait on compute instructions.
# Hoist multi-waits onto standalone InstDrains.
_MULTIWAIT_OK = {
    "InstEventSemaphore",
    "InstBranch",
    "InstCompareAndBranch",
}


_INORDER_ENGINES = {
    mybir.EngineType.PE,
    mybir.EngineType.Activation,
}


def _drop_own_engine_waits(nc):
    """Remove waits on semaphores updated solely by the instruction's own
    engine.  Compute engines issue and retire in order, so same-engine
    WAW/WAR hazards are already ordered; the @complete semaphore ticks lag
    execution by the pipeline depth, so these waits cost ~0.5us each for
    nothing.  DMA/SP waits are kept (queues run in parallel)."""
    f = nc.m.functions[0]
    from collections import defaultdict
    upd = defaultdict(set)
    for blk in f.blocks:
        for inst in blk.instructions:
            si = inst.sync_info
            if si is None:
                continue
            for u in si.on_update:
                upd[u.id].add(inst.engine)
    n = 0
    for blk in f.blocks:
        for inst in blk.instructions:
            si = inst.sync_info
            if si is None or not si.on_wait:
                continue
            if inst.engine in _INORDER_ENGINES:
                # own-engine waits are redundant for in-order PE/ACT
                keep = [
                    w for w in si.on_wait if upd.get(w.id, set()) != {inst.engine}
                ]
            else:
                continue
            if len(keep) != len(si.on_wait):
                n += len(si.on_wait) - len(keep)
                inst.sync_info = mybir.SyncInfo(
                    on_wait=keep, on_update=list(si.on_update)
                )
    return n


def _split_multi_waits(nc):
    f = nc.m.functions[0]
    n_split = 0
    for blk in f.blocks:
        insts = list(blk.instructions)
        out = []
        for inst in insts:
            tn = type(inst).__name__
            si = inst.sync_info
            waits = list(si.on_wait) if si is not None else []
            if len(waits) > 1 and tn not in _MULTIWAIT_OK:
                for w in waits:
                    d = mybir.InstDrain(
                        name=nc.get_next_instruction_name(),
                        ins=[],
                        outs=[],
                        bass_is_fusable=False,
                    )
                    d.engine = inst.engine
                    d.sync_info = mybir.SyncInfo(on_wait=[w], on_update=[])
                    out.append(d)
                inst.sync_info = mybir.SyncInfo(
                    on_wait=[], on_update=list(si.on_update)
                )
                n_split += 1
            out.append(inst)
        blk.instructions = out
    return n_split


_CACHED = {}


def _get_graph():
    if "nc" not in _CACHED:
        _CACHED["nc"] = build_graph()
    return _CACHED["nc"]


def kernel(query, keys, seq_len, W1, b1, W2, b2, W3, b3):
    query = np.asarray(query, dtype=np.float32).reshape(B, F)
    keys = np.asarray(keys, dtype=np.float32)
    seq = np.asarray(seq_len).reshape(B, 1)
    W1 = np.asarray(W1, dtype=np.float32)
    W2 = np.asarray(W2, dtype=np.float32)
    W3 = np.asarray(W3, dtype=np.float32)
    b1 = np.asarray(b1, dtype=np.float32)
    b2 = np.asarray(b2, dtype=np.float32)

    W1q, W1k, W1d, W1m = W1[0:F], W1[F : 2 * F], W1[2 * F : 3 * F], W1[3 * F :]
    Ws = np.concatenate([W1k - W1d, W1m], axis=0).astype(ml_dtypes.bfloat16)
    W2p = np.zeros((H1, 64), np.float32)
    W2p[:, 0:H2] = W2
    W2p = W2p.astype(ml_dtypes.bfloat16)
    W3pp = np.zeros((128, 2), np.float32)
    W3pp[0:H2, 0] = W3[:, 0]
    W3pp[64 : 64 + H2, 1] = W3[:, 0]
    W3pp = W3pp.astype(ml_dtypes.bfloat16)
    b2pp = np.zeros((128, 1), np.float32)
    b2pp[0:H2, 0] = b2
    b2pp[64 : 64 + H2, 0] = b2
    # b3 is constant across s -> softmax-invariant -> dropped

    # per-batch relu1 bias A = q @ (W1q + W1d) + b1, shipped as [H1, B]
    A = (query @ (W1q + W1d) + b1).T.astype(np.float32)
    A = np.ascontiguousarray(A)
    # additive mask: 0 where s < seq_len else NEG_BIG
    amask = np.where(np.arange(S)[None, :] < seq, 0.0, NEG_BIG).astype(np.float32)

    kb = keys.astype(ml_dtypes.bfloat16)          # [B, S, F]
    rhsT = np.empty((128, B, S), dtype=ml_dtypes.bfloat16)
    rhsT[0:F] = kb.transpose(2, 0, 1)
    rhsT[F:128] = (keys * query[:, None, :]).astype(ml_dtypes.bfloat16).transpose(2, 0, 1)

    nc = _get_graph()
    in_maps = []
    for i in range(NCORES):
        lo, hi = i * BPC, (i + 1) * BPC
        in_maps.append(
            {
                "keys": np.ascontiguousarray(kb[lo:hi]),
                "rhsT": np.ascontiguousarray(rhsT[:, lo:hi, :]),
                "Abias": np.ascontiguousarray(A[:, lo:hi]),
                "amask": np.ascontiguousarray(amask[lo:hi]),
                "Ws": Ws,
                "W2p": W2p,
                "W3pp": W3pp,
                "b2pp": b2pp,
            }
        )

    trace = os.environ.get("KERNEL_TRACE") == "1"
    if trace:
        try:
            import antenv.axon_hooks  # noqa: F401  (registered by the test shim)
        except ImportError:
            trace = False
    res = run_bass_kernel_spmd(
        nc, in_maps, core_ids=list(range(NCORES)), trace=trace
    )
    _CACHED["exec_time_ns"] = getattr(res, "exec_time_ns", None)
    _CACHED["profile_json"] = getattr(res, "profile_json", None)
    out = np.concatenate([np.asarray(r["out"]) for r in res.results], axis=0)
    return out.reshape(B, 1, F).astype(np.float32)


if __name__ == "__main__":
    rng = np.random.default_rng(0)
    inputs = {
        "query": rng.standard_normal((B, 1, F), dtype=np.float32),
        "keys": rng.standard_normal((B, S, F), dtype=np.float32),
        "seq_len": rng.integers(0, S, size=(B, 1)).astype(np.int64),
        "W1": rng.standard_normal((4 * F, H1), dtype=np.float32) / 16,
        "b1": np.zeros(H1, np.float32),
        "W2": rng.standard_normal((H1, H2), dtype=np.float32) / 9,
        "b2": np.zeros(H2, np.float32),
        "W3": rng.standard_normal((H2, 1), dtype=np.float32) / 6.3,
        "b3": np.zeros(1, np.float32),
    }
    out = kernel(**inputs)
    print("out", out.shape, out.dtype)

